# revision 10
# baseline (speedup 1.0000x reference)
"""Trainium2 Bass kernel for nn_Attention_77824807403911 (sparse_attention).

Math (per batch element, no softmax => associativity):
    q = x @ Wq^T + bq ; v = x @ Wv^T + bv          [1024, 256]
    rq = rope(q) ; rv = rope(v)
    per head h (16 heads, hd=16):  att_h = rq_h @ (rq_h^T @ rv_h) / 4
    out = att @ Wo^T + bo

Instead of the 1024x1024 score matrix we compute the 16x16 Gram per head
(64x fewer flops), realized as a full 256x256 Gram masked to the
block-diagonal, folded with Wo into a single per-batch [256,256] weight:
    F[e,f]  = sum_s rv[s,e] rq[s,f]       (Gram, transposed blocks)
    BDT     = F .* blockmask
    W2[f,o] = sum_e BDT[e,f] * Wo[o,e] / 4
    outT    = W2^T @ rqT + bo             ([256, 1024])

Sharding: data-parallel over batch, 1 element per core, no collectives.

Schedule notes (v2):
- inputs split into small DMAs across the two HWDGE rings so the first
  projection matmul starts as soon as the first x chunk lands;
- trig tables generated on-chip (8KB DMA of 4 distinct rows + PE
  broadcast matmul) instead of a 512KB DMA;
- pipeline chunked along s (512): proj (PE) -> evict+bias (ACT) ->
  rope (DVE) -> xbar transpose (Sync) -> Gram accumulate (PE);
- output stored bf16 (host upcasts) to halve output traffic.
"""

import numpy as np
import ml_dtypes

import concourse.bass as bass
import concourse.bacc as bacc
import concourse.tile as tile
from concourse import mybir
from concourse.bass_utils import run_bass_kernel_spmd

B, S, D, H, HD = 8, 1024, 256, 16, 16
N_CORES = 8
BF16 = mybir.dt.bfloat16
F32 = mybir.dt.float32

# channel permutation: [evens of pairs 0-63 (theta=1), evens of pairs 64-127
# (theta=1e-4), odds of pairs 0-63, odds of pairs 64-127]
PERM = np.concatenate(
    [np.arange(0, 128, 2), np.arange(128, 256, 2),
     np.arange(1, 128, 2), np.arange(129, 256, 2)]
)


def _host_tables():
    s = np.arange(S, dtype=np.float64) + 1.0
    trig = np.stack([
        np.sin(s), np.cos(s), np.sin(1e-4 * s), np.cos(1e-4 * s)
    ])                                      # [4, 1024]
    sel_sin = np.zeros((4, 128))
    sel_cos = np.zeros((4, 128))
    sel_sin[0, 0:64] = 1.0
    sel_sin[2, 64:128] = 1.0
    sel_cos[1, 0:64] = 1.0
    sel_cos[3, 64:128] = 1.0
    trigB = np.concatenate([trig, sel_sin, sel_cos],
                           axis=1).astype(ml_dtypes.bfloat16)  # [4, 1280]
    a = np.arange(256)
    headp = (a % 128) // 8
    mask = (headp[:, None] == headp[None, :]).astype(ml_dtypes.bfloat16)
    return trigB, mask


def build_kernel():
    nc = bacc.Bacc()
    xT = nc.declare_dram_parameter("xT", [D, S], BF16, isOutput=False)
    # wbig columns: [wqt | wvt | wot | mask], each [256, 256]; then 3 bias cols
    wbig = nc.declare_dram_parameter("wbig", [D, 4 * D + 3], BF16, isOutput=False)
    # trigB rows: sin(s), cos(s), sin(1e-4 s), cos(1e-4 s), s = 1..1024,
    # then sel_sin [4,128] and sel_cos [4,128] broadcast selectors
    trigB = nc.declare_dram_parameter("trigB", [4, S + 256], BF16, isOutput=False)
    outT = nc.declare_dram_parameter("outT", [D, S], BF16, isOutput=True)

    with tile.TileContext(nc) as tc:
        _body(tc, xT, wbig, trigB, outT)
    nc.compile()
    return nc


def _body(tc, xT, wbig, trigB, outT):
    nc = tc.nc
    NS = 2          # s chunks of 512
    SC = S // NS    # 512

    with (
        tc.tile_pool(name="const", bufs=1) as cpool,
        tc.tile_pool(name="acts", bufs=1) as apool,
        tc.tile_pool(name="psum", bufs=4, space="PSUM") as pp,
        tc.tile_pool(name="outp", bufs=4) as opool,
    ):
        # ---- tiny on-chip constants (no DMA deps) ----
        scratch = cpool.tile([128, 512], BF16, tag="scratch", name="scratch")
        nc.gpsimd.memset(scratch[:], 0.25)

        # PE clock-ramp spam while inputs stream in (cheap F=128 matmuls);
        # reuses the trig PSUM buffers (tag trigp) to stay within 8 banks
        for wi in range(8):
            warm_ps = pp.tile([128, 512], F32, tag="trigp", bufs=2,
                              name=f"warm_ps{wi}")
            nc.tensor.matmul(warm_ps[:, 0:128], scratch[:, 0:128],
                             scratch[:, 0:128], start=True, stop=True,
                             skip_group_check=True)

        # ---- input DMAs, split across both HWDGE rings ----
        # trigB_sb cols: [4 trig rows over s | sel_sin [4,128] | sel_cos [4,128]]
        trigB_sb = cpool.tile([4, S + 256], BF16, tag="trigB", name="trigB_sb")
        sel_sin = trigB_sb[:, S:S + 128]
        sel_cos = trigB_sb[:, S + 128:S + 256]
        w_sb = [cpool.tile([128, 4 * D + 3], BF16, tag=f"wbig{cc}",
                           name=f"wbig{cc}") for cc in range(2)]
        xT_sb = [cpool.tile([128, S], BF16, tag=f"xT{cc}", name=f"xT{cc}")
                 for cc in range(2)]
        nc.sync.dma_start(w_sb[0][:], wbig[0:128, :])
        nc.scalar.dma_start(trigB_sb[:], trigB[:])
        nc.scalar.dma_start(w_sb[1][:], wbig[128:256, :])
        nc.sync.dma_start(xT_sb[0][:, 0:SC], xT[0:128, 0:SC])
        nc.scalar.dma_start(xT_sb[1][:, 0:SC], xT[128:256, 0:SC])
        nc.sync.dma_start(xT_sb[0][:, SC:S], xT[0:128, SC:S])
        nc.scalar.dma_start(xT_sb[1][:, SC:S], xT[128:256, SC:S])

        def wslice(idx, cc, col0, ncol):
            return w_sb[cc][:, idx * D + col0: idx * D + col0 + ncol]

        def bias_ap(idx, cc):
            return w_sb[cc][:, 4 * D + idx: 4 * D + idx + 1]

        # ---- trig tables via PE broadcast: [128, 1024] sin | cos ----
        trig_sb = cpool.tile([128, 2 * S], BF16, tag="trig", name="trig_sb")
        for half in range(2):       # 0: sin, 1: cos
            sel = sel_sin if half == 0 else sel_cos
            for sc in range(NS):
                ps = pp.tile([128, SC], F32, tag="trigp", bufs=2, name="trig_ps")
                nc.tensor.matmul(ps[:], sel, trigB_sb[:, sc * SC:(sc + 1) * SC],
                                 start=True, stop=True)
                if half == 0:
                    nc.scalar.activation(
                        trig_sb[:, half * S + sc * SC: half * S + (sc + 1) * SC],
                        ps[:], mybir.ActivationFunctionType.Copy)
                else:
                    nc.vector.tensor_scalar_mul(
                        trig_sb[:, half * S + sc * SC: half * S + (sc + 1) * SC],
                        ps[:], 1.0)
        sin_sb = trig_sb[:, 0:S]
        cos_sb = trig_sb[:, S:2 * S]

        # activations (transposed-permuted layout), 2 partition chunks each
        def act2(tag, width=S, dtype=BF16):
            return [apool.tile([128, width], dtype, tag=f"{tag}{cc}",
                               name=f"{tag}{cc}") for cc in range(2)]

        qT = act2("qT")
        vT = act2("vT")
        rqT = act2("rqT")
        rvT = act2("rvT")
        # natural layout: per sc one [128, 4*256] tile: [s-in-tile, st, chan]
        rq_nat = [apool.tile([128, 4 * D], BF16, tag=f"rqn{sc}", name=f"rqn{sc}")
                  for sc in range(NS)]
        rv_nat = [apool.tile([128, 4 * D], BF16, tag=f"rvn{sc}", name=f"rvn{sc}")
                  for sc in range(NS)]

        # ---- per-chunk projection matmuls (PE) ----
        def proj_chunk(widx, sc, dst_ps):
            for ac in range(2):
                ps = pp.tile([128, SC], F32, tag="mm", bufs=4,
                             name=f"proj_ps_{widx}_{sc}_{ac}")
                for dc in range(2):
                    nc.tensor.matmul(
                        ps[:],
                        wslice(widx, dc, ac * 128, 128),
                        xT_sb[dc][:, sc * SC:(sc + 1) * SC],
                        start=(dc == 0), stop=(dc == 1),
                    )
                dst_ps[ac] = ps

        # PE order: q(sc0), v(sc0), q(sc1), v(sc1)
        q_ps = [[None, None] for _ in range(NS)]
        v_ps = [[None, None] for _ in range(NS)]
        for sc in range(NS):
            proj_chunk(0, sc, q_ps[sc])
            proj_chunk(1, sc, v_ps[sc])

        # ---- evict + bias (ACT), rope (DVE), transpose (Sync) per chunk ----
        def evict(ps, bidx, dstT, ac, sc):
            nc.scalar.activation(
                dstT[ac][:, sc * SC:(sc + 1) * SC], ps[:],
                mybir.ActivationFunctionType.Identity, bias=bias_ap(bidx, ac))

        def rope_chunk(srcT, dstT, sc, tmp_tag):
            sl = slice(sc * SC, (sc + 1) * SC)
            E, O = srcT[0][:, sl], srcT[1][:, sl]
            ssl = sin_sb[:, sl]
            csl = cos_sb[:, sl]
            m1 = opool.tile([128, SC], BF16, tag=tmp_tag + "1", bufs=2)
            m2 = opool.tile([128, SC], BF16, tag=tmp_tag + "2", bufs=2)
            m3 = opool.tile([128, SC], BF16, tag=tmp_tag + "3", bufs=2)
            m4 = opool.tile([128, SC], BF16, tag=tmp_tag + "4", bufs=2)
            nc.vector.tensor_tensor(m1[:], E, ssl, mybir.AluOpType.mult)
            nc.vector.tensor_tensor(m3[:], E, csl, mybir.AluOpType.mult)
            nc.vector.tensor_tensor(m2[:], O, csl, mybir.AluOpType.mult)
            nc.vector.tensor_tensor(m4[:], O, ssl, mybir.AluOpType.mult)
            nc.vector.tensor_tensor(dstT[0][:, sl], m1[:], m2[:],
                                    mybir.AluOpType.subtract)
            nc.vector.tensor_tensor(dstT[1][:, sl], m3[:], m4[:],
                                    mybir.AluOpType.add)

        def transpose_chunk(srcT, nat, sc):
            # nat[p, st, j] = srcT[cc][j, st*128 + p] for st in this chunk
            nat3 = nat[:].rearrange("p (st c) -> p st c", c=D)
            for cc in range(2):
                nc.sync.dma_start(
                    nat3[:, :, cc * 128:(cc + 1) * 128],
                    srcT[cc][:, sc * SC:(sc + 1) * SC], transpose=True)

        # emission order per engine:
        # ACT: qe(sc)ac0, qe(sc)ac1, ve(sc)ac0, ve(sc)ac1 ...
        # DVE: rope q(sc), rope v(sc) ...
        for sc in range(NS):
            evict(q_ps[sc][0], 0, qT, 0, sc)
            evict(q_ps[sc][1], 0, qT, 1, sc)
            evict(v_ps[sc][0], 1, vT, 0, sc)
            evict(v_ps[sc][1], 1, vT, 1, sc)
            rope_chunk(qT, rqT, sc, f"rq{sc}_")
            rope_chunk(vT, rvT, sc, f"rv{sc}_")
            transpose_chunk(rqT, rq_nat[sc], sc)
            transpose_chunk(rvT, rv_nat[sc], sc)

        # ---- Gram: Hm[e, f] = sum_s rv[s, e] rq[s, f] (PE, accumulated) ----
        gram_ps = []
        for ec in range(2):
            gram_ps.append(pp.tile([128, D], F32, tag="sm", bufs=2,
                                   name=f"gram_ps{ec}"))
        for sc in range(NS):
            for ec in range(2):
                for stl in range(4):
                    st = sc * 4 + stl
                    nc.tensor.matmul(
                        gram_ps[ec][:],
                        rv_nat[sc][:, stl * D + ec * 128: stl * D + (ec + 1) * 128],
                        rq_nat[sc][:, stl * D: (stl + 1) * D],
                        start=(st == 0), stop=(st == 7),
                        skip_group_check=True,
                    )

        # ---- mask -> BDT (DVE) ----
        bdt = act2("bdt", width=D)
        for ec in range(2):
            nc.vector.tensor_tensor(
                bdt[ec][:], gram_ps[ec][:], wslice(3, ec, 0, D),
                mybir.AluOpType.mult)

        # ---- W2[f, o] = sum_e BDT[e, f] wot[e, o] / 4 ----
        w2 = act2("w2", width=D)
        for fc in range(2):
            ps = pp.tile([128, D], F32, tag="sm", bufs=2, name=f"w2_ps{fc}")
            for ec in range(2):
                nc.tensor.matmul(
                    ps[:],
                    bdt[ec][:, fc * 128:(fc + 1) * 128],
                    wslice(2, ec, 0, D),
                    start=(ec == 0), stop=(ec == 1),
                )
            nc.vector.tensor_scalar_mul(w2[fc][:], ps[:], 0.25)

        # ---- final: outT[o, s] = sum_f W2[f, o] rqT[f, s] + bo ----
        for oc in range(2):
            for sc in range(NS):
                ps = pp.tile([128, SC], F32, tag="mm", bufs=4,
                             name=f"fin_ps{oc}{sc}")
                for fc in range(2):
                    nc.tensor.matmul(
                        ps[:],
                        w2[fc][:, oc * 128:(oc + 1) * 128],
                        rqT[fc][:, sc * SC:(sc + 1) * SC],
                        start=(fc == 0), stop=(fc == 1),
                    )
                ot = opool.tile([128, SC], BF16, tag="out_sb", bufs=4,
                                name=f"out_sb{oc}{sc}")
                nc.scalar.activation(
                    ot[:], ps[:],
                    mybir.ActivationFunctionType.Identity,
                    bias=bias_ap(2, oc),
                )
                nc.sync.dma_start(
                    outT[oc * 128:(oc + 1) * 128, sc * SC:(sc + 1) * SC], ot[:])


_NC_CACHE = None


def _get_nc():
    global _NC_CACHE
    if _NC_CACHE is None:
        _NC_CACHE = build_kernel()
    return _NC_CACHE


def make_in_maps(x, wq_w, wq_b, wv_w, wv_b, wo_w, wo_b):
    trigB, mask = _host_tables()
    wq_p = np.ascontiguousarray(wq_w[PERM].T).astype(ml_dtypes.bfloat16)   # [d, a]
    wv_p = np.ascontiguousarray(wv_w[PERM].T).astype(ml_dtypes.bfloat16)
    wo_p = np.ascontiguousarray(wo_w[:, PERM].T).astype(ml_dtypes.bfloat16)  # [a(e), o]
    bias3 = np.stack([wq_b[PERM], wv_b[PERM], wo_b], axis=1).astype(ml_dtypes.bfloat16)
    wbig = np.ascontiguousarray(
        np.concatenate([wq_p, wv_p, wo_p, mask, bias3], axis=1))
    in_maps = []
    for b in range(B):
        in_maps.append({
            "xT": np.ascontiguousarray(x[b].T).astype(ml_dtypes.bfloat16),
            "wbig": wbig, "trigB": np.ascontiguousarray(trigB),
        })
    return in_maps


TRACE = False
RUN_KWARGS = {}
LAST_RESULT = None


def kernel(x, wq_w, wq_b, wk_w, wk_b, wv_w, wv_b, wo_w, wo_b):
    global LAST_RESULT
    x = np.asarray(x, dtype=np.float32)
    in_maps = make_in_maps(x, np.asarray(wq_w, np.float32), np.asarray(wq_b, np.float32),
                           np.asarray(wv_w, np.float32), np.asarray(wv_b, np.float32),
                           np.asarray(wo_w, np.float32), np.asarray(wo_b, np.float32))
    nc = _get_nc()
    res = run_bass_kernel_spmd(nc, in_maps, core_ids=list(range(N_CORES)),
                               trace=TRACE, **RUN_KWARGS)
    LAST_RESULT = res
    outs = [np.ascontiguousarray(res.results[b]["outT"].T) for b in range(B)]
    return np.stack(outs).astype(np.float32)
